# revision 2
# baseline (speedup 1.0000x reference)
"""Trainium2 Bass kernel for nn_Coo2FulSimple (periodic pairwise displacement /
squared-distance / cutoff-mask, a.k.a. coo-to-full neighbor expansion).

Contract: kernel(pos, cel, ent, sft_cel) -> (vec, sod, mask), matching
reference():
    pos [B,N,3] f32 fractional, cel [B,3,3] f32, ent [B,N] bool, sft_cel [S,3] i32
    vec  [B,S,N,N,3] f32, sod [B,S,N,N] f32, mask [B,S,N,N] bool

Sharding: each of the 8 NeuronCores handles 41 (s, i-tile-of-128) units of one
batch (cores 0-3 -> b=0, 4-7 -> b=1; 27*6=162 real units per batch + 2 dummy
pad units so all cores run the identical SPMD program).

Per unit, on-device (partition dim = 128 i-atoms, free dim = 768 j-atoms):
    vx,vy,vz = (negXj + Xi) - sx        DVE tensor_scalar (bit-exact ref assoc)
    m_c      = Square(v_c)              ACT (bit-exact fp32 square)
    sod      = (m1+m2)+m3               PE identity-matmul PSUM accumulate (exact)
    sod_sb   = Copy(sod)                ACT PSUM->SBUF
    d2       = Square(sod - 18)         ACT (exact; (d2<324) <=> 0<sod<36)
    maskf    = (sod < t36)              GPSIMD tensor_scalar
    vxm/sodm = (sod < t36) * {vx,sod}   DVE scalar_tensor_tensor
    vym/vzm  = v * maskf                GPSIMD tensor_tensor
    m8       = (d2 < t324) as u8        DVE tensor_scalar
(t36/t324 fold the ent_i mask per-partition; ent_j is folded into negXj; the
self-pair at zero shift is excluded exactly by the two-sided d2 window for the
mask output, while vec/sod products are zero there automatically.)
"""

import numpy as np

B, N, S = 2, 768, 27
P, F = 128, 768
NT = N // P            # 6 i-tiles per batch
REAL = S * NT          # 162 real units per batch
UPC = 41               # units per core (4 cores/batch: 4*41=164 >= 162)
NCORES = 8

_PROG = {}
_LAST_EXEC_NS = None


def _build_program():
    import concourse.bacc as bacc
    import concourse.mybir as mybir
    from concourse.tile import TileContext
    from concourse.alu_op_type import AluOpType

    A = AluOpType
    DT = mybir.dt.float32
    DU8 = mybir.dt.uint8
    SQ = mybir.ActivationFunctionType.Square

    nc = bacc.Bacc()
    negp_in = nc.declare_dram_parameter("negp", [P, 3 * F], DT, isOutput=False)
    par_in = nc.declare_dram_parameter("par", [P, UPC * 8], DT, isOutput=False)
    id_in = nc.declare_dram_parameter("ident", [P, P], DT, isOutput=False)
    vxo = nc.declare_dram_parameter("vxo", [UPC, P, F], DT, isOutput=True)
    vyo = nc.declare_dram_parameter("vyo", [UPC, P, F], DT, isOutput=True)
    vzo = nc.declare_dram_parameter("vzo", [UPC, P, F], DT, isOutput=True)
    sodo = nc.declare_dram_parameter("sodo", [UPC, P, F], DT, isOutput=True)
    msko = nc.declare_dram_parameter("msko", [UPC, P, F], DU8, isOutput=True)

    with TileContext(nc) as tc:
        with (
            tc.tile_pool(name="const", bufs=1) as cpool,
            tc.tile_pool(name="work", bufs=3) as wpool,
            tc.tile_pool(name="outs", bufs=4) as opool,
            tc.tile_pool(name="ps", bufs=4, space="PSUM") as ppool,
        ):
            negp = cpool.tile([P, 3 * F], DT)
            nc.sync.dma_start(out=negp[:], in_=negp_in[:])
            par = cpool.tile([P, UPC * 8], DT)
            nc.sync.dma_start(out=par[:], in_=par_in[:])
            ident = cpool.tile([P, P], DT)
            nc.sync.dma_start(out=ident[:], in_=id_in[:])
            nbias = cpool.tile([P, 1], DT)
            nc.vector.memset(nbias[:], -18.0)

            for u in range(UPC):
                o = u * 8
                xi = par[:, o + 0 : o + 1]
                yi = par[:, o + 1 : o + 2]
                zi = par[:, o + 2 : o + 3]
                sx = par[:, o + 3 : o + 4]
                sy = par[:, o + 4 : o + 5]
                sz = par[:, o + 5 : o + 6]
                t36 = par[:, o + 6 : o + 7]
                t324 = par[:, o + 7 : o + 8]

                vx = wpool.tile([P, F], DT, tag="vx")
                vy = wpool.tile([P, F], DT, tag="vy")
                vz = wpool.tile([P, F], DT, tag="vz")
                nc.vector.tensor_scalar(vx[:], negp[:, 0:F], xi, sx, A.add, A.subtract)
                nc.vector.tensor_scalar(vy[:], negp[:, F : 2 * F], yi, sy, A.add, A.subtract)
                nc.vector.tensor_scalar(vz[:], negp[:, 2 * F : 3 * F], zi, sz, A.add, A.subtract)

                m1 = wpool.tile([P, F], DT, tag="m1")
                m2 = wpool.tile([P, F], DT, tag="m2")
                m3 = wpool.tile([P, F], DT, tag="m3")
                nc.scalar.square(m1[:], vx[:])
                nc.scalar.square(m2[:], vy[:])
                nc.scalar.square(m3[:], vz[:])

                sodp = ppool.tile([P, F], DT, tag="sodp")
                for lo, hi in ((0, 512), (512, 768)):
                    nc.tensor.matmul(sodp[:, lo:hi], ident[:], m1[:, lo:hi], start=True, stop=False)
                    nc.tensor.matmul(sodp[:, lo:hi], ident[:], m2[:, lo:hi], start=False, stop=False)
                    nc.tensor.matmul(sodp[:, lo:hi], ident[:], m3[:, lo:hi], start=False, stop=True)

                sod_sb = wpool.tile([P, F], DT, tag="sod_sb")
                nc.scalar.copy(sod_sb[:], sodp[:])
                d2 = wpool.tile([P, F], DT, tag="d2")
                nc.scalar.activation(d2[:], sodp[:], SQ, bias=nbias[:], scale=1.0)

                maskf = wpool.tile([P, F], DT, tag="maskf")
                nc.gpsimd.tensor_scalar(maskf[:], sod_sb[:], t36, None, A.is_lt)

                vxm = opool.tile([P, F], DT, tag="vxm")
                nc.vector.scalar_tensor_tensor(vxm[:], sod_sb[:], t36, vx[:], A.is_lt, A.mult)
                vym = opool.tile([P, F], DT, tag="vym")
                nc.gpsimd.tensor_tensor(vym[:], vy[:], maskf[:], A.mult)
                vzm = opool.tile([P, F], DT, tag="vzm")
                nc.gpsimd.tensor_tensor(vzm[:], vz[:], maskf[:], A.mult)
                sodm = opool.tile([P, F], DT, tag="sodm")
                nc.vector.scalar_tensor_tensor(sodm[:], sod_sb[:], t36, sod_sb[:], A.is_lt, A.mult)
                m8 = opool.tile([P, F], DU8, tag="m8")
                nc.vector.tensor_scalar(m8[:], d2[:], t324, None, A.is_lt)

                nc.sync.dma_start(out=vxo[u], in_=vxm[:])
                nc.sync.dma_start(out=vyo[u], in_=vym[:])
                nc.sync.dma_start(out=vzo[u], in_=vzm[:])
                nc.sync.dma_start(out=sodo[u], in_=sodm[:])
                nc.sync.dma_start(out=msko[u], in_=m8[:])

    nc.finalize()
    return nc


def _unit_map(core):
    """-> list of (r, s, it) for the UPC units of this core (r may be clamped
    dummy repeats past REAL)."""
    out = []
    for u in range(UPC):
        r = (core % 4) * UPC + u
        rr = min(r, REAL - 1)
        out.append((r, rr // NT, rr % NT))
    return out


def kernel(pos, cel, ent, sft_cel):
    global _LAST_EXEC_NS
    import os
    from concourse.bass_utils import run_bass_kernel_spmd

    pos = np.asarray(pos, dtype=np.float32)
    cel = np.asarray(cel, dtype=np.float32)
    ent_b = np.asarray(ent).astype(bool)
    sft = np.asarray(sft_cel).astype(np.float32)

    # host prep (tiny O(N) marshalling): xyz positions & shift vectors, exactly
    # the reference's einsum contractions evaluated in f32
    pos_xyz = np.einsum("bnc,bcx->bnx", pos, cel).astype(np.float32)      # [B,N,3]
    sft_xyz = np.einsum("sc,bcx->bsx", sft, cel).astype(np.float32)      # [B,S,3]

    BIG = np.float32(1.0e6)
    in_maps = []
    for c in range(NCORES):
        b = 0 if c < 4 else 1
        negp = np.empty((P, 3 * F), np.float32)
        for comp in range(3):
            row = np.where(ent_b[b], -pos_xyz[b, :, comp], BIG).astype(np.float32)
            negp[:, comp * F : (comp + 1) * F] = row[None, :]
        par = np.zeros((P, UPC * 8), np.float32)
        for u, (_, s, it) in enumerate(_unit_map(c)):
            o = u * 8
            sl = slice(it * P, (it + 1) * P)
            par[:, o + 0] = pos_xyz[b, sl, 0]
            par[:, o + 1] = pos_xyz[b, sl, 1]
            par[:, o + 2] = pos_xyz[b, sl, 2]
            par[:, o + 3] = sft_xyz[b, s, 0]
            par[:, o + 4] = sft_xyz[b, s, 1]
            par[:, o + 5] = sft_xyz[b, s, 2]
            par[:, o + 6] = np.where(ent_b[b, sl], np.float32(36.0), np.float32(-1.0))
            par[:, o + 7] = np.where(ent_b[b, sl], np.float32(324.0), np.float32(-1.0))
        in_maps.append({"negp": negp, "par": par, "ident": np.eye(P, dtype=np.float32)})

    if "prog" not in _PROG:
        _PROG["prog"] = _build_program()
    nc = _PROG["prog"]

    trace = bool(os.environ.get("TRNK_TRACE"))
    if trace:
        import importlib.util
        if importlib.util.find_spec("antenv") is None or importlib.util.find_spec(
            "antenv.axon_hooks"
        ) is None:
            trace = False
    res = run_bass_kernel_spmd(nc, in_maps, core_ids=list(range(NCORES)), trace=trace)
    _LAST_EXEC_NS = res.exec_time_ns

    vec = np.zeros((B, S, N, N, 3), np.float32)
    sod = np.zeros((B, S, N, N), np.float32)
    mask = np.zeros((B, S, N, N), np.uint8)
    for c in range(NCORES):
        b = 0 if c < 4 else 1
        r = res.results[c]
        for u, (ru, s, it) in enumerate(_unit_map(c)):
            if ru >= REAL:
                continue
            sl = slice(it * P, (it + 1) * P)
            vec[b, s, sl, :, 0] = r["vxo"][u]
            vec[b, s, sl, :, 1] = r["vyo"][u]
            vec[b, s, sl, :, 2] = r["vzo"][u]
            sod[b, s, sl, :] = r["sodo"][u]
            mask[b, s, sl, :] = r["msko"][u]
    return vec, sod, mask.view(np.bool_)
